# revision 16
# baseline (speedup 1.0000x reference)
"""Trainium2 Bass kernel for nn_GunnarODE: neural CDE with hermite spline control.

Contract: kernel(**inputs) takes FULL unsharded inputs (ts, us, ys, W1, b1,
W2, b2, batch_size) and returns the FULL (B, L, Y) output. Internally shards
the batch across 8 NeuronCores (pure data parallel), runs a Bass/Tile kernel
per core, and reassembles.

Structure (v2 — see kernel_baseline.py for the original):
  - x = concat([t, us]) with unit-spaced knots => dt == 1; spline derivative
    at substep i reduces to dXb_i = s_{k-1} + beta_i * (s_k - s_{k-1}) with
    beta = [0, .8125, 1.25, 1.3125] (alpha_i + beta_i == 1).
  - Slopes are replicated host-side to the 128 vfc rows and streamed as
    (NI, 128, B) so dXb is two cheap Pool/DVE elementwise ops per substep
    (no PE broadcast matmul).
  - State is hpre = W1 @ z held in PSUM (accumulated across all substeps);
    the critical path per substep is tanh -> MM2a -> tanh -> mult -> W1Sel,
    half-split (2 column chunks) for pipelining.
  - Time-channel branch is off-path and packed: vft = W2b @ th computed as
    4 col-tiled (M=16, N=128) matmuls into one (128,128) PSUM strip tile,
    one tanh, then 4 row-tiled (K=16, N=128) matmuls accumulate W1h @ vft
    into hpre concurrently on the PE array.
  - z is reconstructed once per interval via RT = pinv(W1) and DMA'd out.
  - All matmuls fp32: the ODE is chaotic (~1e5 error amplification over the
    512 steps); bf16/tf32-class matmuls measurably fail the 2e-2 budget.
"""
import sys
if '/opt/trn_rl_repo' not in sys.path:
    sys.path.insert(0, '/opt/trn_rl_repo')

import numpy as np

N_CORES = 8
L = 512
B_TOT = 4096
U = 8
Y = 16
H = 128
NI = L - 1            # intervals
HSTEP = 0.25          # dt / SUBSTEPS with dt == 1
B_LOC = B_TOT // N_CORES  # 512
HB = B_LOC // 2

BETA = [0.0, 0.8125, 1.25, 1.3125]

_BUILD_CACHE = {}


def _host_constants(W1, b1, W2, b2):
    """Precompute transposed/permuted constant matrices (host-side, free)."""
    # vfc row r <-> W2 output row (y = r%16, channel c = r//16 + 1)
    rowmap = np.array([(r % 16) * 9 + (r // 16 + 1) for r in range(128)])
    cst = {}
    cst["W1T"] = np.ascontiguousarray(W1.T)                        # (16,128)
    cst["W2aT"] = np.ascontiguousarray(W2[rowmap, :].T)            # (128,128)
    cst["W2bT"] = np.ascontiguousarray(W2[np.arange(16) * 9, :].T)  # (128,16)
    cst["b1c"] = np.ascontiguousarray(b1[:, None])                 # (128,1)
    cst["b2c"] = np.ascontiguousarray(b2[rowmap][:, None])         # (128,1)
    # strip bias: partition 32j+y holds b2t[y] (vft strips at p{0,32,64,96})
    b2t = b2[np.arange(16) * 9]
    b2ts = np.zeros((128, 1), dtype=np.float32)
    for j in range(4):
        b2ts[32 * j:32 * j + 16, 0] = b2t
    cst["b2ts"] = b2ts
    # hpre state update: hpre += (h*W1*Sel^T) @ tmp  [tmp = svfc * dXb]
    w1selt = np.zeros((128, 128), dtype=np.float32)  # [r, j] = h*W1[j, r%16]
    for r in range(128):
        w1selt[r, :] = HSTEP * W1[:, r % 16]
    cst["W1SelT"] = w1selt
    # W1h replicas for 4-way row-tiled time-channel update:
    # rows 32j+y hold h*W1[:, y] (same 16x128 block at each row group)
    w1hrep = np.zeros((128, 128), dtype=np.float32)
    for j in range(4):
        w1hrep[32 * j:32 * j + 16, :] = HSTEP * W1.T
    cst["W1hRep"] = w1hrep
    # output reconstruction: z = pinv(W1) @ hpre  (W1 is 128x16, cond ~2)
    R = np.linalg.pinv(W1.astype(np.float64)).astype(np.float32)   # (16,128)
    cst["RT"] = np.ascontiguousarray(R.T)                          # (128,16)
    return {k: v.astype(np.float32) for k, v in cst.items()}


PACK_VP = False     # col-tiled 4x M=16 time-channel matmuls
PACK_W1H = False    # row-tiled 4x K=16 hpre accumulate
USE_POOL = True    # gpsimd for the slope-diff op


def _build(n_intervals=NI):
    """Build + compile the Bass module (cached per interval count)."""
    key = (n_intervals, PACK_VP, PACK_W1H, USE_POOL)
    if key in _BUILD_CACHE:
        return _BUILD_CACHE[key]

    import concourse.bass as bass
    import concourse.bacc as bacc
    import concourse.tile as tile
    from concourse import mybir

    F32 = mybir.dt.float32
    TANH = mybir.ActivationFunctionType.Tanh
    MULT = mybir.AluOpType.mult
    ADD = mybir.AluOpType.add
    SUB = mybir.AluOpType.subtract

    nc = bacc.Bacc("TRN2", target_bir_lowering=False, debug=False,
                   num_devices=N_CORES)

    d_srep = nc.dram_tensor("srep", (n_intervals, 128, B_LOC), F32, kind="ExternalInput")
    d_ys0 = nc.dram_tensor("ys0T", (16, B_LOC), F32, kind="ExternalInput")
    d_W1T = nc.dram_tensor("W1T", (16, 128), F32, kind="ExternalInput")
    d_W2aT = nc.dram_tensor("W2aT", (128, 128), F32, kind="ExternalInput")
    d_W2bT = nc.dram_tensor("W2bT", (128, 16), F32, kind="ExternalInput")
    d_b1 = nc.dram_tensor("b1c", (128, 1), F32, kind="ExternalInput")
    d_b2c = nc.dram_tensor("b2c", (128, 1), F32, kind="ExternalInput")
    d_b2ts = nc.dram_tensor("b2ts", (128, 1), F32, kind="ExternalInput")
    d_W1SelT = nc.dram_tensor("W1SelT", (128, 128), F32, kind="ExternalInput")
    d_W1hRep = nc.dram_tensor("W1hRep", (128, 128), F32, kind="ExternalInput")
    d_RT = nc.dram_tensor("RT", (128, 16), F32, kind="ExternalInput")
    d_out = nc.dram_tensor("out", (n_intervals, 16, B_LOC), F32, kind="ExternalOutput")

    with tile.TileContext(nc) as tc:
        with (
            tc.tile_pool(name="consts", bufs=1) as consts,
            tc.tile_pool(name="zpool", bufs=2) as zpool,
            tc.tile_pool(name="work", bufs=2) as work,
            tc.tile_pool(name="srp", bufs=3) as srp,
            tc.tile_pool(name="dxp", bufs=2) as dxp,
            tc.tile_pool(name="ps1", bufs=1, space="PSUM") as ps1,
            tc.tile_pool(name="ps2", bufs=2, space="PSUM") as ps2,
            tc.tile_pool(name="ps3", bufs=2, space="PSUM") as ps3,
            tc.tile_pool(name="ps4", bufs=1, space="PSUM") as ps4,
        ):
            W1T = consts.tile([16, 128], F32)
            W2aT = consts.tile([128, 128], F32)
            W2bT = consts.tile([128, 16], F32)
            b1c = consts.tile([128, 1], F32)
            b2c = consts.tile([128, 1], F32)
            b2ts = consts.tile([128, 1], F32)
            W1SelT = consts.tile([128, 128], F32)
            W1hRep = consts.tile([128, 128], F32)
            RT = consts.tile([128, 16], F32)
            nc.sync.dma_start(W1T[:], d_W1T.ap())
            nc.sync.dma_start(W2aT[:], d_W2aT.ap())
            nc.sync.dma_start(W2bT[:], d_W2bT.ap())
            nc.sync.dma_start(b1c[:], d_b1.ap())
            nc.sync.dma_start(b2c[:], d_b2c.ap())
            nc.sync.dma_start(b2ts[:], d_b2ts.ap())
            nc.sync.dma_start(W1SelT[:], d_W1SelT.ap())
            nc.sync.dma_start(W1hRep[:], d_W1hRep.ap())
            nc.sync.dma_start(RT[:], d_RT.ap())

            z0 = zpool.tile([16, B_LOC], F32, tag="z")
            nc.sync.dma_start(z0[:], d_ys0.ap())

            # hpre is THE state: a persistent PSUM accumulator holding W1 @ z.
            hpre = ps1.tile([128, B_LOC], F32, tag="hpre")
            nc.tensor.matmul(hpre[:], W1T[:], z0[:], start=True, stop=False,
                             skip_group_check=True)

            srs = {}

            def load_srep(k):
                if k < n_intervals:
                    t = srp.tile([128, B_LOC], F32, tag="srep", name=f"srep_{k}")
                    nc.sync.dma_start(t[:], d_srep.ap()[k])
                    srs[k] = t

            load_srep(0)
            load_srep(1)
            sprev = srs[0]  # s_{-1} := s_0 (backward-diff init)

            for k in range(n_intervals):
                load_srep(k + 2)
                scur = srs.pop(k)
                # off-path: D = s_k - s_{k-1} (zero for k==0 since sprev is scur)
                D = dxp.tile([128, B_LOC], F32, tag="D")
                eng_d = nc.gpsimd if USE_POOL else nc.vector
                eng_d.tensor_tensor(D[:], scur[:], sprev[:], SUB)
                for i in range(4):
                    if i == 0:
                        dXb = sprev
                    else:
                        dXb = dxp.tile([128, B_LOC], F32, tag="dxb",
                                       name=f"dxb_{k}_{i}")
                        nc.vector.scalar_tensor_tensor(
                            dXb[:], D[:], float(BETA[i]), sprev[:], MULT, ADD)
                    th = work.tile([128, B_LOC], F32, tag="th")
                    svfc = work.tile([128, B_LOC], F32, tag="svfc")
                    tmp = work.tile([128, B_LOC], F32, tag="tmp")
                    vfc_ps = ps2.tile([128, B_LOC], F32, tag="vfc",
                                      name=f"vfc_{k}_{i}")
                    VP = ps3.tile([128, 128], F32, tag="vp", name=f"vp_{k}_{i}")
                    svft = work.tile([128, 128], F32, tag="svft")
                    for h in range(2):
                        sl = slice(h * HB, (h + 1) * HB)
                        nc.scalar.activation(th[:, sl], hpre[:, sl], TANH,
                                             bias=b1c[:])
                        nc.tensor.matmul(vfc_ps[:, sl], W2aT[:], th[:, sl],
                                         start=True, stop=True)
                        nc.scalar.activation(svfc[:, sl], vfc_ps[:, sl], TANH,
                                             bias=b2c[:])
                        nc.vector.tensor_tensor(tmp[:, sl], svfc[:, sl],
                                                dXb[:, sl], MULT)
                    # off-path time-channel branch: 4 col-tiled M=16 matmuls
                    # (strips at p{0,32,64,96}), one tanh, 4 row-tiled K=16
                    # accumulates into hpre.
                    if PACK_VP:
                        for j in range(4):
                            cs = slice(j * 128, (j + 1) * 128)
                            nc.tensor.matmul(VP[32 * j:32 * j + 16, :], W2bT[:],
                                             th[:, cs], start=True, stop=True,
                                             tile_position=(0, 32 * j))
                        nc.scalar.activation(svft[:], VP[:], TANH, bias=b2ts[:])
                    else:
                        vft_ps = ps4.tile([16, B_LOC], F32, tag="vftp",
                                          name=f"vftp_{k}_{i}")
                        nc.tensor.matmul(vft_ps[:], W2bT[:], th[:],
                                         start=True, stop=True)
                        svft_w = work.tile([16, B_LOC], F32, tag="svftw")
                        nc.scalar.activation(svft_w[:], vft_ps[:], TANH,
                                             bias=b2ts[0:16, :])
                    if PACK_VP and PACK_W1H:
                        for j in range(4):
                            cs = slice(j * 128, (j + 1) * 128)
                            nc.tensor.matmul(hpre[:, cs],
                                             W1hRep[32 * j:32 * j + 16, :],
                                             svft[32 * j:32 * j + 16, :],
                                             start=False, stop=False,
                                             skip_group_check=True,
                                             tile_position=(32 * j, 0))
                    elif PACK_VP:
                        # col-strips consumed by 4 plain row-group-0 matmuls
                        for j in range(4):
                            cs = slice(j * 128, (j + 1) * 128)
                            nc.tensor.matmul(hpre[:, cs], W1hRep[0:16, :],
                                             svft[32 * j:32 * j + 16, :],
                                             start=False, stop=False,
                                             skip_group_check=True)
                    else:
                        nc.tensor.matmul(hpre[:], W1hRep[0:16, :], svft_w[:],
                                         start=False, stop=False,
                                         skip_group_check=True)
                    # chain-critical state update
                    for h in range(2):
                        sl = slice(h * HB, (h + 1) * HB)
                        nc.tensor.matmul(hpre[:, sl], W1SelT[:], tmp[:, sl],
                                         start=False, stop=False,
                                         skip_group_check=True)
                # per-interval output: z_{k+1} = pinv(W1) @ hpre
                hps = work.tile([128, B_LOC], F32, tag="hps")
                nc.vector.tensor_copy(hps[:], hpre[:])
                zt_ps = ps4.tile([16, B_LOC], F32, tag="ztp")
                nc.tensor.matmul(zt_ps[:], RT[:], hps[:], start=True, stop=True)
                zout = zpool.tile([16, B_LOC], F32, tag="z", name=f"zout_{k}")
                nc.vector.tensor_copy(zout[:], zt_ps[:])
                nc.sync.dma_start(d_out.ap()[k], zout[:])
                sprev = scur

    nc.compile()
    _BUILD_CACHE[key] = nc
    return nc


def _prep_core_inputs(us, ys, cst, core, n_intervals):
    b0 = core * B_LOC
    usc = us[:, b0:b0 + B_LOC, :]                       # (L, B, U)
    slope = (usc[1:] - usc[:-1]).transpose(0, 2, 1)     # (L-1, U, B)
    srep = np.repeat(slope[:n_intervals], 16, axis=1)   # (NI, 128, B)
    m = {"srep": np.ascontiguousarray(srep, dtype=np.float32),
         "ys0T": np.ascontiguousarray(ys[0, b0:b0 + B_LOC, :].T).astype(np.float32)}
    m.update(cst)
    return m


def kernel(ts, us, ys, W1, b1, W2, b2, batch_size=None, n_intervals=NI):
    from concourse.bass_utils import run_bass_kernel_spmd

    us = np.asarray(us, dtype=np.float32)
    ys = np.asarray(ys, dtype=np.float32)
    cst = _host_constants(np.asarray(W1, np.float32), np.asarray(b1, np.float32),
                          np.asarray(W2, np.float32), np.asarray(b2, np.float32))
    nc = _build(n_intervals)
    in_maps = [_prep_core_inputs(us, ys, cst, c, n_intervals) for c in range(N_CORES)]
    res = run_bass_kernel_spmd(nc, in_maps, core_ids=list(range(N_CORES)))
    out = np.empty((B_TOT, n_intervals + 1, Y), dtype=np.float32)
    out[:, 0, :] = ys[0]
    for c in range(N_CORES):
        b0 = c * B_LOC
        out[b0:b0 + B_LOC, 1:, :] = res.results[c]["out"].transpose(2, 0, 1)
    kernel._last_results = res
    return out
